# revision 27
# baseline (speedup 1.0000x reference)
"""Trainium2 kernel for: out = tanh(x @ scatter_nd(nonzero_ind, kernel_vector, (20000, 4096)) + bias).

Strategy (8 NeuronCores), W-resident / x-streaming, units sharded x8:
  core c owns W[:, c*512:(c+1)*512] (20096 x 512 fp16, SBUF-resident) and
  computes out[:, c*512:(c+1)*512] = x @ W_c for the full batch.

v6 = the baseline's PROVEN DMA steady-state (x stream: 256 KB tiles on
the gpsimd SWDGE queue at ~152 GB/s; W: 128 KB per-k-tile transfers
free-running on the sync/scalar HWDGE rings during chunk 0) plus
targeted startup/boundary/tail fixes that leave it untouched:
  - PE warmup: memset + 8 garbage matmuls at t=0 cover the first-DMA
    wait and the HAM cold-clock window (the PE otherwise starts at
    1.2 GHz and pays ~50% on everything in the first ~3.4us).
  - The first 4 x tiles go on the sync/scalar rings ahead of the W
    flood (ring FIFO: first issued = first done), so the first real
    matmul starts ~2.5us earlier than the SWDGE path allows.
  - Chunks [1024, 512, 512]: PSUM pool A holds the s=0 banks, pool B
    s=1; the 512-chunks alternate pools, so each boundary waits only
    for the first bank set's casts (~0.7us) and the c1->c2 boundary is
    free.  Casts split across Vector AND Scalar engines, A banks first;
    each chunk's four [128,512] results are cast into ONE stage tile
    and leave as two half DMAs -> tail ~4us instead of ~15us.
  - The next chunk's first x pairs are issued on gpsimd BEFORE the
    drain casts (whose matmul sem-waits block the sync/scalar streams).
  - x for the 512-chunks rides as k-tile PAIRS (256 KB -- the transfer
    size the SWDGE queue demonstrably sustains at ~152 GB/s).
"""

import numpy as np

P = 128
B, K, U = 2048, 20000, 4096
USPLIT = 8
KT = 157                 # k-tiles (full contraction per core)
KTP = 158                # padded to even for k-tile pairs
KPAD = KTP * P           # 20224 rows (224 zero pad)
U_SH = U // USPLIT       # 512 unit cols per core
NUS = U_SH // P          # 4 W subtiles (stationary blocks) per k-tile
NXP = KTP // 2           # 79 k-tile pairs for the 512-chunks

# chunk config: list of (batch_size, n_batch_blocks); BBLK = size // nbb = 512
CHUNKS = [(1024, 2), (512, 1), (512, 1)]
BBLK = 512

TRACE = False            # set by test harness for profiled runs
LAST_RESULT = None       # BassKernelResults of the last run (for the harness)

_NC_CACHE = {}


def _build_nc():
    from concourse import bacc
    import concourse.mybir as mybir
    import concourse.tile as tile

    f32 = mybir.dt.float32
    f16 = mybir.dt.float16
    bf16 = mybir.dt.bfloat16

    nc = bacc.Bacc("TRN2", target_bir_lowering=False, debug=False)

    # chunk 0 x tiles: xt0[kt, p, s*512 + b] = x[b, kt*128 + p] (fp16)
    # 512-chunk x pairs: xt{ch}[pair, p, j*512 + b] = x[b0 + b, (2pair+j)*128 + p]
    xt_d = []
    for ch, (bch, nbb) in enumerate(CHUNKS):
        if ch == 0:
            xt_d.append(nc.dram_tensor("xt0", [KT, P, bch], f16,
                                       kind="ExternalInput").ap())
        else:
            xt_d.append(nc.dram_tensor(f"xt{ch}", [NXP, P, 2 * bch], f16,
                                       kind="ExternalInput").ap())
    # W head: k-tiles 0-15 as four 4-k-tile quads (the per-k-tile trickle
    # is lane-paced at ~1.1us/k-tile -- too slow for the first k-tiles);
    # W tail: per k-tile 16..156.
    wh_d = nc.dram_tensor("w_head", [4, P, 4 * U_SH], f16,
                          kind="ExternalInput").ap()
    w_d = nc.dram_tensor("w_sh", [KT - 16, P, U_SH], f16,
                         kind="ExternalInput").ap()
    # out per chunk: o{ch}[p, us, s*BBLK + b] = z^T[us*128+p, s*BBLK+b]
    o_d = [
        nc.dram_tensor(f"o{ch}", [P, NUS, nbb * BBLK], bf16,
                       kind="ExternalOutput").ap()
        for ch, (bch, nbb) in enumerate(CHUNKS)
    ]

    with tile.TileContext(nc) as tc:
        with (
            tc.tile_pool(name="resid", bufs=1) as respool,
            tc.tile_pool(name="xpool", bufs=10) as xpool,
            tc.tile_pool(name="stage", bufs=1) as spool,
            tc.tile_pool(name="warm", bufs=1) as wmpool,
            tc.tile_pool(name="psumA", bufs=1, space="PSUM") as psumA,
            tc.tile_pool(name="psumB", bufs=1, space="PSUM") as psumB,
        ):
            prefetched = {}

            def x_issue(ch, seg, eng=None):
                # ch 0: seg = k-tile index; ch>0: seg = k-tile pair index.
                # The SWDGE queue alone runs at 97% utilization against the
                # x demand, so a fraction of the stream rotates onto the
                # HWDGE rings: 1/4 in chunk 0 (rings also carry the W
                # trickle), 1/3 in later chunks (rings otherwise idle).
                t = xpool.tile([P, 1024], f16, tag="xs", name="xs")
                if eng is None:
                    if ch == 0:
                        eng = {3: nc.sync, 7: nc.scalar}.get(
                            seg % 8, nc.gpsimd)
                    else:
                        eng = {2: nc.sync, 5: nc.scalar}.get(
                            seg % 6, nc.gpsimd)
                eng.dma_start(t[:], xt_d[ch][seg])
                prefetched[(ch, seg)] = t

            # --- PE warmup: keep the PE busy through the HAM cold window
            # while the first x/W DMAs are in flight.
            wtile = wmpool.tile([P, BBLK], f16, tag="wm", name="wm")
            nc.vector.memset(wtile[:], 0.0)
            wps = psumA.tile([P, BBLK], f32, tag="pa0", name="wps")

            whead = [
                respool.tile([P, 4 * U_SH], f16, tag=f"wh{g}", name=f"wh{g}")
                for g in range(4)
            ]
            wres = [
                respool.tile([P, U_SH], f16, tag=f"w{kt}", name=f"w{kt}")
                for kt in range(16, KT)
            ]

            # first 4 x tiles ride the HWDGE rings ahead of the W flood,
            # then the W head quads (k-tiles 0-15).
            x_issue(0, 0, nc.sync)
            x_issue(0, 1, nc.scalar)
            x_issue(0, 2, nc.sync)
            x_issue(0, 3, nc.scalar)
            for g in range(4):
                (nc.sync if g % 2 == 0 else nc.scalar).dma_start(
                    whead[g][:], wh_d[g])

            for _ in range(8):
                nc.tensor.matmul(wps[:], wtile[:, 0:P], wtile[:],
                                 start=True, stop=True)

            n512 = 0
            for ch, (bch, nbb) in enumerate(CHUNKS):
                # PSUM: s=0 blocks in pool A, s=1 in pool B (nbb=2);
                # nbb=1 chunks alternate pools A, B, A, ...
                if nbb == 2:
                    pools = [(psumA, "a"), (psumB, "b")]
                else:
                    pools = [[(psumA, "a"), (psumB, "b")][n512 % 2]]
                    n512 += 1
                psums = [
                    [pool.tile([P, BBLK], f32, tag=f"p{pc}{us}",
                               name=f"ps{ch}_{us}_{s}")
                     for s, (pool, pc) in enumerate(pools)]
                    for us in range(NUS)
                ]

                for kt in range(KT):
                    if ch == 0:
                        if kt >= 16:
                            # W tail free-runs on the HWDGE rings (proven
                            # ~115 GB/s combined; fully resident by ~180us)
                            weng = nc.sync if kt % 2 == 0 else nc.scalar
                            weng.dma_start(wres[kt - 16][:], w_d[kt - 16])
                        seg, col0 = kt, 0
                    else:
                        seg, col0 = kt // 2, (kt % 2) * BBLK
                    if (ch, seg) not in prefetched:
                        x_issue(ch, seg)
                    xs = prefetched[(ch, seg)]
                    if ch == 0 or kt % 2 == 1 or kt == KT - 1:
                        del prefetched[(ch, seg)]  # last use of this tile
                    # first k-tile after a boundary: touch the banks in
                    # the order the previous chunk's casts free them
                    us_order = [0, 2, 1, 3] if (kt == 0 and ch > 0) \
                        else range(NUS)
                    if kt < 16:
                        wsl = whead[kt // 4][:, (kt % 4) * U_SH:
                                             (kt % 4 + 1) * U_SH]
                    else:
                        wsl = wres[kt - 16][:]
                    for us in us_order:
                        for s in range(nbb):
                            nc.tensor.matmul(
                                psums[us][s][:],
                                wsl[:, us * P:(us + 1) * P],
                                xs[:, col0 + s * BBLK:
                                       col0 + (s + 1) * BBLK],
                                start=(kt == 0),
                                stop=(kt == KT - 1),
                            )

                # Prefetch next chunk's first x pairs on gpsimd BEFORE the
                # drain -- the casts block sync/scalar on matmul sems, and
                # the gpsimd stream has no such waits.
                if ch + 1 < len(CHUNKS):
                    for pp in range(3):
                        x_issue(ch + 1, pp)

                # Drain: cast PSUM -> one stage tile; vector does us 0-1,
                # scalar does us 2-3 in parallel; pool-A banks (which the
                # next chunk needs first) are cast before pool-B banks.
                st = spool.tile([P, NUS * nbb * BBLK], bf16,
                                tag="st0" if ch == 0 else "st_s",
                                name=f"st{ch}")
                for s in range(nbb):          # s=0 (pool A) first
                    for us in range(NUS):
                        dst = st[:, (us * nbb + s) * BBLK:
                                 (us * nbb + s + 1) * BBLK]
                        if us < 2:
                            nc.vector.tensor_copy(dst, psums[us][s][:])
                        else:
                            nc.scalar.copy(dst, psums[us][s][:])
                half = 2 * nbb * BBLK
                nc.sync.dma_start(o_d[ch][:, 0:2], st[:, :half])
                nc.scalar.dma_start(o_d[ch][:, 2:4], st[:, half:])

    nc.compile()
    return nc


def _get_nc():
    if "nc" not in _NC_CACHE:
        _NC_CACHE["nc"] = _build_nc()
    return _NC_CACHE["nc"]


def kernel(x, kernel_vector, bias, nonzero_ind):
    global LAST_RESULT
    from concourse.bass_utils import run_bass_kernel_spmd

    x = np.asarray(x, dtype=np.float32)
    kernel_vector = np.asarray(kernel_vector, dtype=np.float32)
    bias = np.asarray(bias, dtype=np.float32)
    nonzero_ind = np.asarray(nonzero_ind)

    nc = _get_nc()

    # Host scatter: dense weights [KT*P, U] fp16 (rows >= 20000 stay zero).
    rows = nonzero_ind[:, 0].astype(np.int64)
    cols = nonzero_ind[:, 1].astype(np.int64)
    w_full = np.zeros(KT * P * U, np.float32)
    np.add.at(w_full, rows * U + cols, kernel_vector)
    w_full = w_full.reshape(KT * P, U).astype(np.float16)

    # x^T padded to KPAD rows, fp16.
    x16 = x.astype(np.float16)
    xt = np.zeros((KPAD, B), np.float16)
    xt[:K] = x16.T

    xt_chunks = []
    b0 = 0
    for ch, (bch, nbb) in enumerate(CHUNKS):
        if ch == 0:
            xc = xt[:KT * P, b0:b0 + bch].reshape(KT, P, bch)
            xt_chunks.append(np.ascontiguousarray(xc))
        else:
            xc = (xt[:, b0:b0 + bch].reshape(NXP, 2, P, bch)
                  .transpose(0, 2, 1, 3))
            xt_chunks.append(np.ascontiguousarray(xc.reshape(NXP, P, 2 * bch)))
        b0 += bch

    in_maps = []
    for c in range(USPLIT):
        wc = w_full[:, c * U_SH:(c + 1) * U_SH]
        w_head = np.ascontiguousarray(
            wc[:16 * P].reshape(4, 4, P, U_SH).transpose(0, 2, 1, 3)
            .reshape(4, P, 4 * U_SH))
        w_sh = np.ascontiguousarray(
            wc[16 * P:].reshape(KT - 16, P, U_SH))
        m = {"w_sh": w_sh, "w_head": w_head}
        for ch in range(len(CHUNKS)):
            m[f"xt{ch}"] = xt_chunks[ch]
        in_maps.append(m)

    kwargs = {}
    if TRACE:
        kwargs = dict(trace=True, trace_cores=list(range(8)))
    res = run_bass_kernel_spmd(nc, in_maps, core_ids=list(range(8)), **kwargs)
    LAST_RESULT = res

    out = np.empty((B, U), np.float32)
    for c in range(USPLIT):
        b0 = 0
        for ch, (bch, nbb) in enumerate(CHUNKS):
            # [P, NUS, nbb*BBLK] -> [nbb, BBLK, NUS, P] -> [bch, U_SH]
            blk = (
                res.results[c][f"o{ch}"]
                .astype(np.float32)
                .reshape(P, NUS, nbb, BBLK)
                .transpose(2, 3, 1, 0)
                .reshape(bch, U_SH)
            )
            out[b0:b0 + bch, c * U_SH:(c + 1) * U_SH] = blk
            b0 += bch
    out += bias[None, :]
    np.tanh(out, out=out)
    return out


# revision 28
# speedup vs baseline: 1.0917x; 1.0917x over previous
"""Trainium2 kernel for: out = tanh(x @ scatter_nd(nonzero_ind, kernel_vector, (20000, 4096)) + bias).

Strategy (8 NeuronCores), W-resident / x-streaming, units sharded x8:
  core c owns W[:, c*512:(c+1)*512] (20096 x 512 fp16, SBUF-resident) and
  computes out[:, c*512:(c+1)*512] = x @ W_c for the full batch.

v6 = the baseline's PROVEN DMA steady-state (x stream: 256 KB tiles on
the gpsimd SWDGE queue at ~152 GB/s; W: 128 KB per-k-tile transfers
free-running on the sync/scalar HWDGE rings during chunk 0) plus
targeted startup/boundary/tail fixes that leave it untouched:
  - PE warmup: memset + 8 garbage matmuls at t=0 cover the first-DMA
    wait and the HAM cold-clock window (the PE otherwise starts at
    1.2 GHz and pays ~50% on everything in the first ~3.4us).
  - The first 4 x tiles go on the sync/scalar rings ahead of the W
    flood (ring FIFO: first issued = first done), so the first real
    matmul starts ~2.5us earlier than the SWDGE path allows.
  - Chunks [1024, 512, 512]: PSUM pool A holds the s=0 banks, pool B
    s=1; the 512-chunks alternate pools, so each boundary waits only
    for the first bank set's casts (~0.7us) and the c1->c2 boundary is
    free.  Casts split across Vector AND Scalar engines, A banks first;
    each chunk's four [128,512] results are cast into ONE stage tile
    and leave as two half DMAs -> tail ~4us instead of ~15us.
  - The next chunk's first x pairs are issued on gpsimd BEFORE the
    drain casts (whose matmul sem-waits block the sync/scalar streams).
  - x for the 512-chunks rides as k-tile PAIRS (256 KB -- the transfer
    size the SWDGE queue demonstrably sustains at ~152 GB/s).
"""

import numpy as np

P = 128
B, K, U = 2048, 20000, 4096
USPLIT = 8
KT = 157                 # k-tiles (full contraction per core)
KTP = 158                # padded to even for k-tile pairs
KPAD = KTP * P           # 20224 rows (224 zero pad)
U_SH = U // USPLIT       # 512 unit cols per core
NUS = U_SH // P          # 4 W subtiles (stationary blocks) per k-tile
NXP = KTP // 2           # 79 k-tile pairs for the 512-chunks

# chunk config: list of (batch_size, n_batch_blocks); BBLK = size // nbb = 512
CHUNKS = [(1024, 2), (512, 1), (512, 1)]
BBLK = 512

TRACE = False            # set by test harness for profiled runs
LAST_RESULT = None       # BassKernelResults of the last run (for the harness)

_NC_CACHE = {}


def _build_nc():
    from concourse import bacc
    import concourse.mybir as mybir
    import concourse.tile as tile

    f32 = mybir.dt.float32
    f16 = mybir.dt.float16
    bf16 = mybir.dt.bfloat16

    nc = bacc.Bacc("TRN2", target_bir_lowering=False, debug=False)

    # chunk 0 x tiles: xt0[kt, p, s*512 + b] = x[b, kt*128 + p] (fp16)
    # 512-chunk x pairs: xt{ch}[pair, p, j*512 + b] = x[b0 + b, (2pair+j)*128 + p]
    xt_d = []
    for ch, (bch, nbb) in enumerate(CHUNKS):
        if ch == 0:
            xt_d.append(nc.dram_tensor("xt0", [KT, P, bch], f16,
                                       kind="ExternalInput").ap())
        else:
            xt_d.append(nc.dram_tensor(f"xt{ch}", [NXP, P, 2 * bch], f16,
                                       kind="ExternalInput").ap())
    # W head: k-tiles 0-15 as four 4-k-tile quads (the per-k-tile trickle
    # is lane-paced at ~1.1us/k-tile -- too slow for the first k-tiles);
    # W tail: per k-tile 16..156.
    wh_d = nc.dram_tensor("w_head", [4, P, 4 * U_SH], f16,
                          kind="ExternalInput").ap()
    w_d = nc.dram_tensor("w_sh", [KT - 16, P, U_SH], f16,
                         kind="ExternalInput").ap()
    # out per chunk: o{ch}[p, us, s*BBLK + b] = z^T[us*128+p, s*BBLK+b]
    o_d = [
        nc.dram_tensor(f"o{ch}", [P, NUS, nbb * BBLK], bf16,
                       kind="ExternalOutput").ap()
        for ch, (bch, nbb) in enumerate(CHUNKS)
    ]

    with tile.TileContext(nc) as tc:
        with (
            tc.tile_pool(name="resid", bufs=1) as respool,
            tc.tile_pool(name="xpool", bufs=10) as xpool,
            tc.tile_pool(name="stage", bufs=1) as spool,
            tc.tile_pool(name="warm", bufs=1) as wmpool,
            tc.tile_pool(name="psumA", bufs=1, space="PSUM") as psumA,
            tc.tile_pool(name="psumB", bufs=1, space="PSUM") as psumB,
        ):
            prefetched = {}

            def x_issue(ch, seg, eng=None):
                # ch 0: seg = k-tile index; ch>0: seg = k-tile pair index.
                # The steady x stream lives EXCLUSIVELY on the gpsimd SWDGE
                # queue: every attempt to rotate part of it onto the HWDGE
                # rings (which carry W / outs) starved the PE -- x tiles
                # get stuck behind ring-FIFO backlogs and ~10us completion
                # -semaphore lane chains.
                t = xpool.tile([P, 1024], f16, tag="xs", name="xs")
                (eng or nc.gpsimd).dma_start(t[:], xt_d[ch][seg])
                prefetched[(ch, seg)] = t

            # --- PE warmup: keep the PE busy through the HAM cold window
            # while the first x/W DMAs are in flight.
            wtile = wmpool.tile([P, BBLK], f16, tag="wm", name="wm")
            nc.vector.memset(wtile[:], 0.0)
            wps = psumA.tile([P, BBLK], f32, tag="pa0", name="wps")

            whead = [
                respool.tile([P, 4 * U_SH], f16, tag=f"wh{g}", name=f"wh{g}")
                for g in range(4)
            ]
            wres = [
                respool.tile([P, U_SH], f16, tag=f"w{kt}", name=f"w{kt}")
                for kt in range(16, KT)
            ]

            # first 4 x tiles ride the HWDGE rings ahead of the W flood,
            # then the W head quads (k-tiles 0-15).
            x_issue(0, 0, nc.sync)
            x_issue(0, 1, nc.scalar)
            x_issue(0, 2, nc.sync)
            x_issue(0, 3, nc.scalar)
            for g in range(4):
                (nc.sync if g % 2 == 0 else nc.scalar).dma_start(
                    whead[g][:], wh_d[g])

            for _ in range(8):
                nc.tensor.matmul(wps[:], wtile[:, 0:P], wtile[:],
                                 start=True, stop=True)

            n512 = 0
            for ch, (bch, nbb) in enumerate(CHUNKS):
                # PSUM: s=0 blocks in pool A, s=1 in pool B (nbb=2);
                # nbb=1 chunks alternate pools A, B, A, ...
                if nbb == 2:
                    pools = [(psumA, "a"), (psumB, "b")]
                else:
                    pools = [[(psumA, "a"), (psumB, "b")][n512 % 2]]
                    n512 += 1
                psums = [
                    [pool.tile([P, BBLK], f32, tag=f"p{pc}{us}",
                               name=f"ps{ch}_{us}_{s}")
                     for s, (pool, pc) in enumerate(pools)]
                    for us in range(NUS)
                ]

                for kt in range(KT):
                    if ch == 0:
                        if kt >= 16:
                            # W tail free-runs on the HWDGE rings (proven
                            # ~115 GB/s combined; fully resident by ~180us)
                            weng = nc.sync if kt % 2 == 0 else nc.scalar
                            weng.dma_start(wres[kt - 16][:], w_d[kt - 16])
                        seg, col0 = kt, 0
                    else:
                        seg, col0 = kt // 2, (kt % 2) * BBLK
                    if (ch, seg) not in prefetched:
                        x_issue(ch, seg)
                    xs = prefetched[(ch, seg)]
                    if ch == 0 or kt % 2 == 1 or kt == KT - 1:
                        del prefetched[(ch, seg)]  # last use of this tile
                    # first k-tile after a boundary: touch the banks in
                    # the order the previous chunk's casts free them
                    us_order = [0, 2, 1, 3] if (kt == 0 and ch > 0) \
                        else range(NUS)
                    if kt < 16:
                        wsl = whead[kt // 4][:, (kt % 4) * U_SH:
                                             (kt % 4 + 1) * U_SH]
                    else:
                        wsl = wres[kt - 16][:]
                    for us in us_order:
                        for s in range(nbb):
                            nc.tensor.matmul(
                                psums[us][s][:],
                                wsl[:, us * P:(us + 1) * P],
                                xs[:, col0 + s * BBLK:
                                       col0 + (s + 1) * BBLK],
                                start=(kt == 0),
                                stop=(kt == KT - 1),
                            )

                # Prefetch next chunk's first x pairs on gpsimd BEFORE the
                # drain -- the casts block sync/scalar on matmul sems, and
                # the gpsimd stream has no such waits.
                if ch + 1 < len(CHUNKS):
                    for pp in range(3):
                        x_issue(ch + 1, pp)

                # Drain: cast PSUM -> one stage tile; vector does us 0-1,
                # scalar does us 2-3 in parallel; pool-A banks (which the
                # next chunk needs first) are cast before pool-B banks.
                st = spool.tile([P, NUS * nbb * BBLK], bf16,
                                tag="st0" if ch == 0 else "st_s",
                                name=f"st{ch}")
                for s in range(nbb):          # s=0 (pool A) first
                    for us in range(NUS):
                        dst = st[:, (us * nbb + s) * BBLK:
                                 (us * nbb + s + 1) * BBLK]
                        if us < 2:
                            nc.vector.tensor_copy(dst, psums[us][s][:])
                        else:
                            nc.scalar.copy(dst, psums[us][s][:])
                half = 2 * nbb * BBLK
                nc.sync.dma_start(o_d[ch][:, 0:2], st[:, :half])
                nc.scalar.dma_start(o_d[ch][:, 2:4], st[:, half:])

    nc.compile()
    return nc


def _get_nc():
    if "nc" not in _NC_CACHE:
        _NC_CACHE["nc"] = _build_nc()
    return _NC_CACHE["nc"]


def kernel(x, kernel_vector, bias, nonzero_ind):
    global LAST_RESULT
    from concourse.bass_utils import run_bass_kernel_spmd

    x = np.asarray(x, dtype=np.float32)
    kernel_vector = np.asarray(kernel_vector, dtype=np.float32)
    bias = np.asarray(bias, dtype=np.float32)
    nonzero_ind = np.asarray(nonzero_ind)

    nc = _get_nc()

    # Host scatter: dense weights [KT*P, U] fp16 (rows >= 20000 stay zero).
    rows = nonzero_ind[:, 0].astype(np.int64)
    cols = nonzero_ind[:, 1].astype(np.int64)
    w_full = np.zeros(KT * P * U, np.float32)
    np.add.at(w_full, rows * U + cols, kernel_vector)
    w_full = w_full.reshape(KT * P, U).astype(np.float16)

    # x^T padded to KPAD rows, fp16.
    x16 = x.astype(np.float16)
    xt = np.zeros((KPAD, B), np.float16)
    xt[:K] = x16.T

    xt_chunks = []
    b0 = 0
    for ch, (bch, nbb) in enumerate(CHUNKS):
        if ch == 0:
            xc = xt[:KT * P, b0:b0 + bch].reshape(KT, P, bch)
            xt_chunks.append(np.ascontiguousarray(xc))
        else:
            xc = (xt[:, b0:b0 + bch].reshape(NXP, 2, P, bch)
                  .transpose(0, 2, 1, 3))
            xt_chunks.append(np.ascontiguousarray(xc.reshape(NXP, P, 2 * bch)))
        b0 += bch

    in_maps = []
    for c in range(USPLIT):
        wc = w_full[:, c * U_SH:(c + 1) * U_SH]
        w_head = np.ascontiguousarray(
            wc[:16 * P].reshape(4, 4, P, U_SH).transpose(0, 2, 1, 3)
            .reshape(4, P, 4 * U_SH))
        w_sh = np.ascontiguousarray(
            wc[16 * P:].reshape(KT - 16, P, U_SH))
        m = {"w_sh": w_sh, "w_head": w_head}
        for ch in range(len(CHUNKS)):
            m[f"xt{ch}"] = xt_chunks[ch]
        in_maps.append(m)

    kwargs = {}
    if TRACE:
        kwargs = dict(trace=True, trace_cores=list(range(8)))
    res = run_bass_kernel_spmd(nc, in_maps, core_ids=list(range(8)), **kwargs)
    LAST_RESULT = res

    out = np.empty((B, U), np.float32)
    for c in range(USPLIT):
        b0 = 0
        for ch, (bch, nbb) in enumerate(CHUNKS):
            # [P, NUS, nbb*BBLK] -> [nbb, BBLK, NUS, P] -> [bch, U_SH]
            blk = (
                res.results[c][f"o{ch}"]
                .astype(np.float32)
                .reshape(P, NUS, nbb, BBLK)
                .transpose(2, 3, 1, 0)
                .reshape(bch, U_SH)
            )
            out[b0:b0 + bch, c * U_SH:(c + 1) * U_SH] = blk
            b0 += bch
    out += bias[None, :]
    np.tanh(out, out=out)
    return out


# revision 33
# speedup vs baseline: 1.0971x; 1.0049x over previous
"""Trainium2 kernel for: out = tanh(x @ scatter_nd(nonzero_ind, kernel_vector, (20000, 4096)) + bias).

Strategy (8 NeuronCores), W-resident / x-streaming, units sharded x8:
  core c owns W[:, c*512:(c+1)*512] (20096 x 512 fp16, SBUF-resident) and
  computes out[:, c*512:(c+1)*512] = x @ W_c for the full batch.

v6 = the baseline's PROVEN DMA steady-state (x stream: 256 KB tiles on
the gpsimd SWDGE queue at ~152 GB/s; W: 128 KB per-k-tile transfers
free-running on the sync/scalar HWDGE rings during chunk 0) plus
targeted startup/boundary/tail fixes that leave it untouched:
  - PE warmup: memset + 8 garbage matmuls at t=0 cover the first-DMA
    wait and the HAM cold-clock window (the PE otherwise starts at
    1.2 GHz and pays ~50% on everything in the first ~3.4us).
  - The first 4 x tiles go on the sync/scalar rings ahead of the W
    flood (ring FIFO: first issued = first done), so the first real
    matmul starts ~2.5us earlier than the SWDGE path allows.
  - Chunks [1024, 512, 512]: PSUM pool A holds the s=0 banks, pool B
    s=1; the 512-chunks alternate pools, so each boundary waits only
    for the first bank set's casts (~0.7us) and the c1->c2 boundary is
    free.  Casts split across Vector AND Scalar engines, A banks first;
    each chunk's four [128,512] results are cast into ONE stage tile
    and leave as two half DMAs -> tail ~4us instead of ~15us.
  - The next chunk's first x pairs are issued on gpsimd BEFORE the
    drain casts (whose matmul sem-waits block the sync/scalar streams).
  - x for the 512-chunks rides as k-tile PAIRS (256 KB -- the transfer
    size the SWDGE queue demonstrably sustains at ~152 GB/s).
"""

import numpy as np

P = 128
B, K, U = 2048, 20000, 4096
USPLIT = 8
KT = 157                 # k-tiles (full contraction per core)
KTP = 158                # padded to even for k-tile pairs
KPAD = KTP * P           # 20224 rows (224 zero pad)
U_SH = U // USPLIT       # 512 unit cols per core
NUS = U_SH // P          # 4 W subtiles (stationary blocks) per k-tile
NXP = KTP // 2           # 79 k-tile pairs for the 512-chunks

# chunk config: list of (batch_size, n_batch_blocks); BBLK = size // nbb = 512
CHUNKS = [(1024, 2), (512, 1), (512, 1)]
BBLK = 512

TRACE = False            # set by test harness for profiled runs
LAST_RESULT = None       # BassKernelResults of the last run (for the harness)

_NC_CACHE = {}


def _build_nc():
    from concourse import bacc
    import concourse.mybir as mybir
    import concourse.tile as tile

    f32 = mybir.dt.float32
    f16 = mybir.dt.float16
    bf16 = mybir.dt.bfloat16

    nc = bacc.Bacc("TRN2", target_bir_lowering=False, debug=False)

    # chunk 0 x tiles: xt0[kt, p, s*512 + b] = x[b, kt*128 + p] (fp16)
    # 512-chunk x pairs: xt{ch}[pair, p, j*512 + b] = x[b0 + b, (2pair+j)*128 + p]
    xt_d = []
    for ch, (bch, nbb) in enumerate(CHUNKS):
        if ch == 0:
            xt_d.append(nc.dram_tensor("xt0", [KT, P, bch], f16,
                                       kind="ExternalInput").ap())
        else:
            xt_d.append(nc.dram_tensor(f"xt{ch}", [NXP, P, 2 * bch], f16,
                                       kind="ExternalInput").ap())
    # W head (k-tiles 0-15): the per-k-tile trickle is lane-paced at
    # ~1.1us/k-tile -- too slow for the first k-tiles -- and the fabric
    # ramps slowly for the first ~15us, so the very first k-tiles ride as
    # two small pairs (arrive ~9.5us) and k-tiles 4-15 as three quads.
    # W tail: per k-tile 16..156.
    whp_d = nc.dram_tensor("w_hp", [2, P, 2 * U_SH], f16,
                           kind="ExternalInput").ap()
    whq_d = nc.dram_tensor("w_hq", [3, P, 4 * U_SH], f16,
                           kind="ExternalInput").ap()
    w_d = nc.dram_tensor("w_sh", [KT - 16, P, U_SH], f16,
                         kind="ExternalInput").ap()
    # out per chunk: o{ch}[p, us, s*BBLK + b] = z^T[us*128+p, s*BBLK+b]
    o_d = [
        nc.dram_tensor(f"o{ch}", [P, NUS, nbb * BBLK], bf16,
                       kind="ExternalOutput").ap()
        for ch, (bch, nbb) in enumerate(CHUNKS)
    ]

    with tile.TileContext(nc) as tc:
        with (
            tc.tile_pool(name="resid", bufs=1) as respool,
            tc.tile_pool(name="xpool", bufs=8) as xpool,
            tc.tile_pool(name="stage", bufs=1) as spool,
            tc.tile_pool(name="warm", bufs=1) as wmpool,
            tc.tile_pool(name="psumA", bufs=1, space="PSUM") as psumA,
            tc.tile_pool(name="psumB", bufs=1, space="PSUM") as psumB,
        ):
            prefetched = {}

            def x_issue(ch, seg, eng=None):
                # ch 0: seg = k-tile index; ch>0: seg = k-tile pair index.
                # The steady x stream lives EXCLUSIVELY on the gpsimd SWDGE
                # queue: every attempt to rotate part of it onto the HWDGE
                # rings (which carry W / outs) starved the PE -- x tiles
                # get stuck behind ring-FIFO backlogs and ~10us completion
                # -semaphore lane chains.
                t = xpool.tile([P, 1024], f16, tag="xs", name="xs")
                (eng or nc.gpsimd).dma_start(t[:], xt_d[ch][seg])
                prefetched[(ch, seg)] = t

            # --- PE warmup: keep the PE busy through the HAM cold window
            # while the first x/W DMAs are in flight.
            wtile = wmpool.tile([P, BBLK], f16, tag="wm", name="wm")
            nc.vector.memset(wtile[:], 0.0)
            wps = psumA.tile([P, BBLK], f32, tag="pa0", name="wps")

            whp = [
                respool.tile([P, 2 * U_SH], f16, tag=f"whp{g}", name=f"whp{g}")
                for g in range(2)
            ]
            whq = [
                respool.tile([P, 4 * U_SH], f16, tag=f"whq{g}", name=f"whq{g}")
                for g in range(3)
            ]
            wres = [
                respool.tile([P, U_SH], f16, tag=f"w{kt}", name=f"w{kt}")
                for kt in range(16, KT)
            ]

            # Startup, ring-FIFO ordered: x tile 0 and the k-tile-0/1 W
            # pair lead each ring so the first matmul can start ~9.5us;
            # the W flood stays strictly behind them.
            x_issue(0, 0, nc.sync)
            x_issue(0, 1, nc.scalar)
            nc.sync.dma_start(whp[0][:], whp_d[0])
            nc.scalar.dma_start(whp[1][:], whp_d[1])
            x_issue(0, 2, nc.sync)
            x_issue(0, 3, nc.scalar)
            nc.sync.dma_start(whq[0][:], whq_d[0])
            nc.scalar.dma_start(whq[1][:], whq_d[1])
            nc.sync.dma_start(whq[2][:], whq_d[2])

            for _ in range(8):
                nc.tensor.matmul(wps[:], wtile[:, 0:P], wtile[:],
                                 start=True, stop=True)

            n512 = 0
            for ch, (bch, nbb) in enumerate(CHUNKS):
                # PSUM: s=0 blocks in pool A, s=1 in pool B (nbb=2);
                # nbb=1 chunks alternate pools A, B, A, ...
                if nbb == 2:
                    pools = [(psumA, "a"), (psumB, "b")]
                else:
                    pools = [[(psumA, "a"), (psumB, "b")][n512 % 2]]
                    n512 += 1
                psums = [
                    [pool.tile([P, BBLK], f32, tag=f"p{pc}{us}",
                               name=f"ps{ch}_{us}_{s}")
                     for s, (pool, pc) in enumerate(pools)]
                    for us in range(NUS)
                ]

                for kt in range(KT):
                    if ch == 0:
                        if kt >= 16:
                            # W tail free-runs on the HWDGE rings (proven
                            # ~115 GB/s combined; fully resident by ~180us)
                            weng = nc.sync if kt % 2 == 0 else nc.scalar
                            weng.dma_start(wres[kt - 16][:], w_d[kt - 16])
                        seg, col0 = kt, 0
                    else:
                        seg, col0 = kt // 2, (kt % 2) * BBLK
                    if (ch, seg) not in prefetched:
                        x_issue(ch, seg)
                    xs = prefetched[(ch, seg)]
                    if ch == 0 or kt % 2 == 1 or kt == KT - 1:
                        del prefetched[(ch, seg)]  # last use of this tile
                    # first k-tile after a boundary: touch the banks in
                    # the order the previous chunk's casts free them
                    us_order = [0, 2, 1, 3] if (kt == 0 and ch > 0) \
                        else range(NUS)
                    if kt < 4:
                        wsl = whp[kt // 2][:, (kt % 2) * U_SH:
                                           (kt % 2 + 1) * U_SH]
                    elif kt < 16:
                        g, jj = (kt - 4) // 4, (kt - 4) % 4
                        wsl = whq[g][:, jj * U_SH:(jj + 1) * U_SH]
                    else:
                        wsl = wres[kt - 16][:]
                    for us in us_order:
                        for s in range(nbb):
                            nc.tensor.matmul(
                                psums[us][s][:],
                                wsl[:, us * P:(us + 1) * P],
                                xs[:, col0 + s * BBLK:
                                       col0 + (s + 1) * BBLK],
                                start=(kt == 0),
                                stop=(kt == KT - 1),
                            )

                # Prefetch next chunk's first x pairs on gpsimd BEFORE the
                # drain -- the casts block sync/scalar on matmul sems, and
                # the gpsimd stream has no such waits.
                if ch + 1 < len(CHUNKS):
                    for pp in range(3):
                        x_issue(ch + 1, pp)

                # Drain: cast PSUM -> one stage tile; vector does us 0-1,
                # scalar does us 2-3 in parallel; pool-A banks (which the
                # next chunk needs first) are cast before pool-B banks.
                st = spool.tile([P, NUS * nbb * BBLK], bf16,
                                tag="st0" if ch == 0 else "st_s",
                                name=f"st{ch}")
                for s in range(nbb):          # s=0 (pool A) first
                    for us in range(NUS):
                        dst = st[:, (us * nbb + s) * BBLK:
                                 (us * nbb + s + 1) * BBLK]
                        if us < 2:
                            nc.vector.tensor_copy(dst, psums[us][s][:])
                        else:
                            nc.scalar.copy(dst, psums[us][s][:])
                half = 2 * nbb * BBLK
                nc.sync.dma_start(o_d[ch][:, 0:2], st[:, :half])
                nc.scalar.dma_start(o_d[ch][:, 2:4], st[:, half:])

    nc.compile()
    return nc


def _get_nc():
    if "nc" not in _NC_CACHE:
        _NC_CACHE["nc"] = _build_nc()
    return _NC_CACHE["nc"]


def kernel(x, kernel_vector, bias, nonzero_ind):
    global LAST_RESULT
    from concourse.bass_utils import run_bass_kernel_spmd

    x = np.asarray(x, dtype=np.float32)
    kernel_vector = np.asarray(kernel_vector, dtype=np.float32)
    bias = np.asarray(bias, dtype=np.float32)
    nonzero_ind = np.asarray(nonzero_ind)

    nc = _get_nc()

    # Host scatter: dense weights [KT*P, U] fp16 (rows >= 20000 stay zero).
    rows = nonzero_ind[:, 0].astype(np.int64)
    cols = nonzero_ind[:, 1].astype(np.int64)
    w_full = np.zeros(KT * P * U, np.float32)
    np.add.at(w_full, rows * U + cols, kernel_vector)
    w_full = w_full.reshape(KT * P, U).astype(np.float16)

    # x^T padded to KPAD rows, fp16.
    x16 = x.astype(np.float16)
    xt = np.zeros((KPAD, B), np.float16)
    xt[:K] = x16.T

    xt_chunks = []
    b0 = 0
    for ch, (bch, nbb) in enumerate(CHUNKS):
        if ch == 0:
            xc = xt[:KT * P, b0:b0 + bch].reshape(KT, P, bch)
            xt_chunks.append(np.ascontiguousarray(xc))
        else:
            xc = (xt[:, b0:b0 + bch].reshape(NXP, 2, P, bch)
                  .transpose(0, 2, 1, 3))
            xt_chunks.append(np.ascontiguousarray(xc.reshape(NXP, P, 2 * bch)))
        b0 += bch

    in_maps = []
    for c in range(USPLIT):
        wc = w_full[:, c * U_SH:(c + 1) * U_SH]
        w_hp = np.ascontiguousarray(
            wc[:4 * P].reshape(2, 2, P, U_SH).transpose(0, 2, 1, 3)
            .reshape(2, P, 2 * U_SH))
        w_hq = np.ascontiguousarray(
            wc[4 * P:16 * P].reshape(3, 4, P, U_SH).transpose(0, 2, 1, 3)
            .reshape(3, P, 4 * U_SH))
        w_sh = np.ascontiguousarray(
            wc[16 * P:].reshape(KT - 16, P, U_SH))
        m = {"w_sh": w_sh, "w_hp": w_hp, "w_hq": w_hq}
        for ch in range(len(CHUNKS)):
            m[f"xt{ch}"] = xt_chunks[ch]
        in_maps.append(m)

    kwargs = {}
    if TRACE:
        kwargs = dict(trace=True, trace_cores=list(range(8)))
    res = run_bass_kernel_spmd(nc, in_maps, core_ids=list(range(8)), **kwargs)
    LAST_RESULT = res

    out = np.empty((B, U), np.float32)
    for c in range(USPLIT):
        b0 = 0
        for ch, (bch, nbb) in enumerate(CHUNKS):
            # [P, NUS, nbb*BBLK] -> [nbb, BBLK, NUS, P] -> [bch, U_SH]
            blk = (
                res.results[c][f"o{ch}"]
                .astype(np.float32)
                .reshape(P, NUS, nbb, BBLK)
                .transpose(2, 3, 1, 0)
                .reshape(bch, U_SH)
            )
            out[b0:b0 + bch, c * U_SH:(c + 1) * U_SH] = blk
            b0 += bch
    out += bias[None, :]
    np.tanh(out, out=out)
    return out


# revision 35
# speedup vs baseline: 1.1001x; 1.0027x over previous
"""Trainium2 kernel for: out = tanh(x @ scatter_nd(nonzero_ind, kernel_vector, (20000, 4096)) + bias).

Strategy (8 NeuronCores), W-resident / x-streaming, units sharded x8:
  core c owns W[:, c*512:(c+1)*512] (20096 x 512 fp16, SBUF-resident) and
  computes out[:, c*512:(c+1)*512] = x @ W_c for the full batch.

v6 = the baseline's PROVEN DMA steady-state (x stream: 256 KB tiles on
the gpsimd SWDGE queue at ~152 GB/s; W: 128 KB per-k-tile transfers
free-running on the sync/scalar HWDGE rings during chunk 0) plus
targeted startup/boundary/tail fixes that leave it untouched:
  - PE warmup: memset + 8 garbage matmuls at t=0 cover the first-DMA
    wait and the HAM cold-clock window (the PE otherwise starts at
    1.2 GHz and pays ~50% on everything in the first ~3.4us).
  - The first 4 x tiles go on the sync/scalar rings ahead of the W
    flood (ring FIFO: first issued = first done), so the first real
    matmul starts ~2.5us earlier than the SWDGE path allows.
  - Chunks [1024, 512, 512]: PSUM pool A holds the s=0 banks, pool B
    s=1; the 512-chunks alternate pools, so each boundary waits only
    for the first bank set's casts (~0.7us) and the c1->c2 boundary is
    free.  Casts split across Vector AND Scalar engines, A banks first;
    each chunk's four [128,512] results are cast into ONE stage tile
    and leave as two half DMAs -> tail ~4us instead of ~15us.
  - The next chunk's first x pairs are issued on gpsimd BEFORE the
    drain casts (whose matmul sem-waits block the sync/scalar streams).
  - x for the 512-chunks rides as k-tile PAIRS (256 KB -- the transfer
    size the SWDGE queue demonstrably sustains at ~152 GB/s).
"""

import numpy as np

P = 128
B, K, U = 2048, 20000, 4096
USPLIT = 8
KT = 157                 # k-tiles (full contraction per core)
KTP = 158                # padded to even for k-tile pairs
KPAD = KTP * P           # 20224 rows (224 zero pad)
U_SH = U // USPLIT       # 512 unit cols per core
NUS = U_SH // P          # 4 W subtiles (stationary blocks) per k-tile
NXP = KTP // 2           # 79 k-tile pairs for the 512-chunks

# chunk config: list of (batch_size, n_batch_blocks); BBLK = size // nbb = 512
CHUNKS = [(1024, 2), (512, 1), (512, 1)]
BBLK = 512

TRACE = False            # set by test harness for profiled runs
LAST_RESULT = None       # BassKernelResults of the last run (for the harness)

_NC_CACHE = {}


def _build_nc():
    from concourse import bacc
    import concourse.mybir as mybir
    import concourse.tile as tile

    f32 = mybir.dt.float32
    f16 = mybir.dt.float16
    bf16 = mybir.dt.bfloat16

    nc = bacc.Bacc("TRN2", target_bir_lowering=False, debug=False)

    # chunk 0 x tiles: xt0[kt, p, s*512 + b] = x[b, kt*128 + p] (fp16)
    # 512-chunk x pairs: xt{ch}[pair, p, j*512 + b] = x[b0 + b, (2pair+j)*128 + p]
    xt_d = []
    for ch, (bch, nbb) in enumerate(CHUNKS):
        if ch == 0:
            xt_d.append(nc.dram_tensor("xt0", [KT, P, bch], f16,
                                       kind="ExternalInput").ap())
        else:
            xt_d.append(nc.dram_tensor(f"xt{ch}", [NXP, P, 2 * bch], f16,
                                       kind="ExternalInput").ap())
    # W head (k-tiles 0-15): the per-k-tile trickle is lane-paced at
    # ~1.1us/k-tile -- too slow for the first k-tiles -- and the fabric
    # ramps slowly for the first ~15us, so the very first k-tiles ride as
    # two small pairs (arrive ~9.5us) and k-tiles 4-15 as three quads.
    # W tail: per k-tile 16..156.
    whp_d = nc.dram_tensor("w_hp", [2, P, 2 * U_SH], f16,
                           kind="ExternalInput").ap()
    whq_d = nc.dram_tensor("w_hq", [3, P, 4 * U_SH], f16,
                           kind="ExternalInput").ap()
    w_d = nc.dram_tensor("w_sh", [KT - 16, P, U_SH], f16,
                         kind="ExternalInput").ap()
    # out per chunk: o{ch}[p, us, s*BBLK + b] = z^T[us*128+p, s*BBLK+b]
    o_d = [
        nc.dram_tensor(f"o{ch}", [P, NUS, nbb * BBLK], bf16,
                       kind="ExternalOutput").ap()
        for ch, (bch, nbb) in enumerate(CHUNKS)
    ]

    with tile.TileContext(nc) as tc:
        with (
            tc.tile_pool(name="resid", bufs=1) as respool,
            tc.tile_pool(name="xpool", bufs=8) as xpool,
            tc.tile_pool(name="stage", bufs=1) as spool,
            tc.tile_pool(name="warm", bufs=1) as wmpool,
            tc.tile_pool(name="psumA", bufs=1, space="PSUM") as psumA,
            tc.tile_pool(name="psumB", bufs=1, space="PSUM") as psumB,
        ):
            prefetched = {}

            def x_issue(ch, seg, eng=None):
                # ch 0: seg = k-tile index; ch>0: seg = k-tile pair index.
                # The steady x stream lives EXCLUSIVELY on the gpsimd SWDGE
                # queue: every attempt to rotate part of it onto the HWDGE
                # rings (which carry W / outs) starved the PE -- x tiles
                # get stuck behind ring-FIFO backlogs and ~10us completion
                # -semaphore lane chains.
                t = xpool.tile([P, 1024], f16, tag="xs", name="xs")
                (eng or nc.gpsimd).dma_start(t[:], xt_d[ch][seg])
                prefetched[(ch, seg)] = t

            # --- PE warmup: keep the PE busy through the HAM cold window
            # while the first x/W DMAs are in flight.
            wtile = wmpool.tile([P, BBLK], f16, tag="wm", name="wm")
            nc.vector.memset(wtile[:], 0.0)
            wps = psumA.tile([P, BBLK], f32, tag="pa0", name="wps")

            whp = [
                respool.tile([P, 2 * U_SH], f16, tag=f"whp{g}", name=f"whp{g}")
                for g in range(2)
            ]
            whq = [
                respool.tile([P, 4 * U_SH], f16, tag=f"whq{g}", name=f"whq{g}")
                for g in range(3)
            ]
            wres = [
                respool.tile([P, U_SH], f16, tag=f"w{kt}", name=f"w{kt}")
                for kt in range(16, KT)
            ]

            # Startup, ring-FIFO ordered: x tile 0 and the k-tile-0/1 W
            # pair lead each ring so the first matmul can start ~9.5us;
            # the W flood stays strictly behind them.
            x_issue(0, 0, nc.sync)
            x_issue(0, 1, nc.scalar)
            nc.sync.dma_start(whp[0][:], whp_d[0])
            nc.scalar.dma_start(whp[1][:], whp_d[1])
            x_issue(0, 2, nc.sync)
            x_issue(0, 3, nc.scalar)
            nc.sync.dma_start(whq[0][:], whq_d[0])
            nc.scalar.dma_start(whq[1][:], whq_d[1])
            x_issue(0, 4, nc.sync)
            x_issue(0, 5, nc.scalar)
            nc.sync.dma_start(whq[2][:], whq_d[2])

            for _ in range(8):
                nc.tensor.matmul(wps[:], wtile[:, 0:P], wtile[:],
                                 start=True, stop=True)

            n512 = 0
            for ch, (bch, nbb) in enumerate(CHUNKS):
                # PSUM: s=0 blocks in pool A, s=1 in pool B (nbb=2);
                # nbb=1 chunks alternate pools A, B, A, ...
                if nbb == 2:
                    pools = [(psumA, "a"), (psumB, "b")]
                else:
                    pools = [[(psumA, "a"), (psumB, "b")][n512 % 2]]
                    n512 += 1
                psums = [
                    [pool.tile([P, BBLK], f32, tag=f"p{pc}{us}",
                               name=f"ps{ch}_{us}_{s}")
                     for s, (pool, pc) in enumerate(pools)]
                    for us in range(NUS)
                ]

                for kt in range(KT):
                    if ch == 0:
                        if kt >= 16:
                            # W tail free-runs on the HWDGE rings (proven
                            # ~115 GB/s combined; fully resident by ~180us)
                            weng = nc.sync if kt % 2 == 0 else nc.scalar
                            weng.dma_start(wres[kt - 16][:], w_d[kt - 16])
                        seg, col0 = kt, 0
                    else:
                        seg, col0 = kt // 2, (kt % 2) * BBLK
                    if (ch, seg) not in prefetched:
                        x_issue(ch, seg)
                    xs = prefetched[(ch, seg)]
                    if ch == 0 or kt % 2 == 1 or kt == KT - 1:
                        del prefetched[(ch, seg)]  # last use of this tile
                    # first k-tile after a boundary: touch the banks in
                    # the order the previous chunk's casts free them
                    us_order = [0, 2, 1, 3] if (kt == 0 and ch > 0) \
                        else range(NUS)
                    if kt < 4:
                        wsl = whp[kt // 2][:, (kt % 2) * U_SH:
                                           (kt % 2 + 1) * U_SH]
                    elif kt < 16:
                        g, jj = (kt - 4) // 4, (kt - 4) % 4
                        wsl = whq[g][:, jj * U_SH:(jj + 1) * U_SH]
                    else:
                        wsl = wres[kt - 16][:]
                    for us in us_order:
                        for s in range(nbb):
                            nc.tensor.matmul(
                                psums[us][s][:],
                                wsl[:, us * P:(us + 1) * P],
                                xs[:, col0 + s * BBLK:
                                       col0 + (s + 1) * BBLK],
                                start=(kt == 0),
                                stop=(kt == KT - 1),
                            )

                # Prefetch next chunk's first x pairs on gpsimd BEFORE the
                # drain -- the casts block sync/scalar on matmul sems, and
                # the gpsimd stream has no such waits.
                if ch + 1 < len(CHUNKS):
                    for pp in range(3):
                        x_issue(ch + 1, pp)

                # Drain: cast PSUM -> one stage tile; vector does us 0-1,
                # scalar does us 2-3 in parallel; pool-A banks (which the
                # next chunk needs first) are cast before pool-B banks.
                st = spool.tile([P, NUS * nbb * BBLK], bf16,
                                tag="st0" if ch == 0 else "st_s",
                                name=f"st{ch}")
                for s in range(nbb):          # s=0 (pool A) first
                    for us in range(NUS):
                        dst = st[:, (us * nbb + s) * BBLK:
                                 (us * nbb + s + 1) * BBLK]
                        if us < 2:
                            nc.vector.tensor_copy(dst, psums[us][s][:])
                        else:
                            nc.scalar.copy(dst, psums[us][s][:])
                if ch == len(CHUNKS) - 1:
                    # final chunk: quarter DMAs, each right after its cast,
                    # so the last (smallest) transfer finishes sooner
                    qw = nbb * BBLK
                    for us in range(NUS):
                        eng = nc.sync if us < 2 else nc.scalar
                        eng.dma_start(o_d[ch][:, us:us + 1],
                                      st[:, us * qw:(us + 1) * qw])
                else:
                    half = 2 * nbb * BBLK
                    nc.sync.dma_start(o_d[ch][:, 0:2], st[:, :half])
                    nc.scalar.dma_start(o_d[ch][:, 2:4], st[:, half:])

    nc.compile()
    return nc


def _get_nc():
    if "nc" not in _NC_CACHE:
        _NC_CACHE["nc"] = _build_nc()
    return _NC_CACHE["nc"]


def kernel(x, kernel_vector, bias, nonzero_ind):
    global LAST_RESULT
    from concourse.bass_utils import run_bass_kernel_spmd

    x = np.asarray(x, dtype=np.float32)
    kernel_vector = np.asarray(kernel_vector, dtype=np.float32)
    bias = np.asarray(bias, dtype=np.float32)
    nonzero_ind = np.asarray(nonzero_ind)

    nc = _get_nc()

    # Host scatter: dense weights [KT*P, U] fp16 (rows >= 20000 stay zero).
    rows = nonzero_ind[:, 0].astype(np.int64)
    cols = nonzero_ind[:, 1].astype(np.int64)
    w_full = np.zeros(KT * P * U, np.float32)
    np.add.at(w_full, rows * U + cols, kernel_vector)
    w_full = w_full.reshape(KT * P, U).astype(np.float16)

    # x^T padded to KPAD rows, fp16.
    x16 = x.astype(np.float16)
    xt = np.zeros((KPAD, B), np.float16)
    xt[:K] = x16.T

    xt_chunks = []
    b0 = 0
    for ch, (bch, nbb) in enumerate(CHUNKS):
        if ch == 0:
            xc = xt[:KT * P, b0:b0 + bch].reshape(KT, P, bch)
            xt_chunks.append(np.ascontiguousarray(xc))
        else:
            xc = (xt[:, b0:b0 + bch].reshape(NXP, 2, P, bch)
                  .transpose(0, 2, 1, 3))
            xt_chunks.append(np.ascontiguousarray(xc.reshape(NXP, P, 2 * bch)))
        b0 += bch

    in_maps = []
    for c in range(USPLIT):
        wc = w_full[:, c * U_SH:(c + 1) * U_SH]
        w_hp = np.ascontiguousarray(
            wc[:4 * P].reshape(2, 2, P, U_SH).transpose(0, 2, 1, 3)
            .reshape(2, P, 2 * U_SH))
        w_hq = np.ascontiguousarray(
            wc[4 * P:16 * P].reshape(3, 4, P, U_SH).transpose(0, 2, 1, 3)
            .reshape(3, P, 4 * U_SH))
        w_sh = np.ascontiguousarray(
            wc[16 * P:].reshape(KT - 16, P, U_SH))
        m = {"w_sh": w_sh, "w_hp": w_hp, "w_hq": w_hq}
        for ch in range(len(CHUNKS)):
            m[f"xt{ch}"] = xt_chunks[ch]
        in_maps.append(m)

    kwargs = {}
    if TRACE:
        kwargs = dict(trace=True, trace_cores=list(range(8)))
    res = run_bass_kernel_spmd(nc, in_maps, core_ids=list(range(8)), **kwargs)
    LAST_RESULT = res

    out = np.empty((B, U), np.float32)
    for c in range(USPLIT):
        b0 = 0
        for ch, (bch, nbb) in enumerate(CHUNKS):
            # [P, NUS, nbb*BBLK] -> [nbb, BBLK, NUS, P] -> [bch, U_SH]
            blk = (
                res.results[c][f"o{ch}"]
                .astype(np.float32)
                .reshape(P, NUS, nbb, BBLK)
                .transpose(2, 3, 1, 0)
                .reshape(bch, U_SH)
            )
            out[b0:b0 + bch, c * U_SH:(c + 1) * U_SH] = blk
            b0 += bch
    out += bias[None, :]
    np.tanh(out, out=out)
    return out


# revision 36
# speedup vs baseline: 1.1020x; 1.0017x over previous
"""Trainium2 kernel for: out = tanh(x @ scatter_nd(nonzero_ind, kernel_vector, (20000, 4096)) + bias).

Strategy (8 NeuronCores), W-resident / x-streaming, units sharded x8:
  core c owns W[:, c*512:(c+1)*512] (20096 x 512 fp16, SBUF-resident) and
  computes out[:, c*512:(c+1)*512] = x @ W_c for the full batch.

v6 = the baseline's PROVEN DMA steady-state (x stream: 256 KB tiles on
the gpsimd SWDGE queue at ~152 GB/s; W: 128 KB per-k-tile transfers
free-running on the sync/scalar HWDGE rings during chunk 0) plus
targeted startup/boundary/tail fixes that leave it untouched:
  - PE warmup: memset + 8 garbage matmuls at t=0 cover the first-DMA
    wait and the HAM cold-clock window (the PE otherwise starts at
    1.2 GHz and pays ~50% on everything in the first ~3.4us).
  - The first 4 x tiles go on the sync/scalar rings ahead of the W
    flood (ring FIFO: first issued = first done), so the first real
    matmul starts ~2.5us earlier than the SWDGE path allows.
  - Chunks [1024, 512, 512]: PSUM pool A holds the s=0 banks, pool B
    s=1; the 512-chunks alternate pools, so each boundary waits only
    for the first bank set's casts (~0.7us) and the c1->c2 boundary is
    free.  Casts split across Vector AND Scalar engines, A banks first;
    each chunk's four [128,512] results are cast into ONE stage tile
    and leave as two half DMAs -> tail ~4us instead of ~15us.
  - The next chunk's first x pairs are issued on gpsimd BEFORE the
    drain casts (whose matmul sem-waits block the sync/scalar streams).
  - x for the 512-chunks rides as k-tile PAIRS (256 KB -- the transfer
    size the SWDGE queue demonstrably sustains at ~152 GB/s).
"""

import numpy as np

P = 128
B, K, U = 2048, 20000, 4096
USPLIT = 8
KT = 157                 # k-tiles (full contraction per core)
KTP = 158                # padded to even for k-tile pairs
KPAD = KTP * P           # 20224 rows (224 zero pad)
U_SH = U // USPLIT       # 512 unit cols per core
NUS = U_SH // P          # 4 W subtiles (stationary blocks) per k-tile
NXP = KTP // 2           # 79 k-tile pairs for the 512-chunks

# chunk config: list of (batch_size, n_batch_blocks); BBLK = size // nbb = 512
CHUNKS = [(1024, 2), (512, 1), (512, 1)]
BBLK = 512

TRACE = False            # set by test harness for profiled runs
LAST_RESULT = None       # BassKernelResults of the last run (for the harness)

_NC_CACHE = {}


def _build_nc():
    from concourse import bacc
    import concourse.mybir as mybir
    import concourse.tile as tile

    f32 = mybir.dt.float32
    f16 = mybir.dt.float16
    bf16 = mybir.dt.bfloat16

    nc = bacc.Bacc("TRN2", target_bir_lowering=False, debug=False)

    # chunk 0 x tiles: xt0[kt, p, s*512 + b] = x[b, kt*128 + p] (fp16)
    # 512-chunk x pairs: xt{ch}[pair, p, j*512 + b] = x[b0 + b, (2pair+j)*128 + p]
    xt_d = []
    for ch, (bch, nbb) in enumerate(CHUNKS):
        if ch == 0:
            xt_d.append(nc.dram_tensor("xt0", [KT, P, bch], f16,
                                       kind="ExternalInput").ap())
        else:
            xt_d.append(nc.dram_tensor(f"xt{ch}", [NXP, P, 2 * bch], f16,
                                       kind="ExternalInput").ap())
    # W head (k-tiles 0-15): the per-k-tile trickle is lane-paced at
    # ~1.1us/k-tile -- too slow for the first k-tiles -- and the fabric
    # ramps slowly for the first ~15us, so the very first k-tiles ride as
    # two small pairs (arrive ~9.5us) and k-tiles 4-15 as three quads.
    # W tail: per k-tile 16..156.
    whp_d = nc.dram_tensor("w_hp", [2, P, 2 * U_SH], f16,
                           kind="ExternalInput").ap()
    whq_d = nc.dram_tensor("w_hq", [3, P, 4 * U_SH], f16,
                           kind="ExternalInput").ap()
    w_d = nc.dram_tensor("w_sh", [KT - 16, P, U_SH], f16,
                         kind="ExternalInput").ap()
    # out per chunk: o{ch}[p, us, s*BBLK + b] = z^T[us*128+p, s*BBLK+b]
    o_d = [
        nc.dram_tensor(f"o{ch}", [P, NUS, nbb * BBLK], bf16,
                       kind="ExternalOutput").ap()
        for ch, (bch, nbb) in enumerate(CHUNKS)
    ]

    with tile.TileContext(nc) as tc:
        with (
            tc.tile_pool(name="resid", bufs=1) as respool,
            tc.tile_pool(name="xpool", bufs=8) as xpool,
            tc.tile_pool(name="stage", bufs=1) as spool,
            tc.tile_pool(name="warm", bufs=1) as wmpool,
            tc.tile_pool(name="psumA", bufs=1, space="PSUM") as psumA,
            tc.tile_pool(name="psumB", bufs=1, space="PSUM") as psumB,
        ):
            prefetched = {}

            def x_issue(ch, seg, eng=None):
                # ch 0: seg = k-tile index; ch>0: seg = k-tile pair index.
                # The steady x stream lives EXCLUSIVELY on the gpsimd SWDGE
                # queue: every attempt to rotate part of it onto the HWDGE
                # rings (which carry W / outs) starved the PE -- x tiles
                # get stuck behind ring-FIFO backlogs and ~10us completion
                # -semaphore lane chains.
                t = xpool.tile([P, 1024], f16, tag="xs", name="xs")
                (eng or nc.gpsimd).dma_start(t[:], xt_d[ch][seg])
                prefetched[(ch, seg)] = t

            # --- PE warmup: keep the PE busy through the HAM cold window
            # while the first x/W DMAs are in flight.
            wtile = wmpool.tile([P, BBLK], f16, tag="wm", name="wm")
            nc.vector.memset(wtile[:], 0.0)
            wps = psumA.tile([P, BBLK], f32, tag="pa0", name="wps")

            whp = [
                respool.tile([P, 2 * U_SH], f16, tag=f"whp{g}", name=f"whp{g}")
                for g in range(2)
            ]
            whq = [
                respool.tile([P, 4 * U_SH], f16, tag=f"whq{g}", name=f"whq{g}")
                for g in range(3)
            ]
            wres = [
                respool.tile([P, U_SH], f16, tag=f"w{kt}", name=f"w{kt}")
                for kt in range(16, KT)
            ]

            # Startup, ring-FIFO ordered: x tile 0 and the k-tile-0/1 W
            # pair lead each ring so the first matmul can start ~9.5us;
            # the W flood stays strictly behind them.
            x_issue(0, 0, nc.sync)
            x_issue(0, 1, nc.scalar)
            nc.sync.dma_start(whp[0][:], whp_d[0])
            nc.scalar.dma_start(whp[1][:], whp_d[1])
            x_issue(0, 2, nc.sync)
            x_issue(0, 3, nc.scalar)
            nc.sync.dma_start(whq[0][:], whq_d[0])
            nc.scalar.dma_start(whq[1][:], whq_d[1])
            x_issue(0, 4, nc.sync)
            x_issue(0, 5, nc.scalar)
            nc.sync.dma_start(whq[2][:], whq_d[2])
            x_issue(0, 6, nc.scalar)
            x_issue(0, 7, nc.sync)

            for _ in range(8):
                nc.tensor.matmul(wps[:], wtile[:, 0:P], wtile[:],
                                 start=True, stop=True)

            n512 = 0
            for ch, (bch, nbb) in enumerate(CHUNKS):
                # PSUM: s=0 blocks in pool A, s=1 in pool B (nbb=2);
                # nbb=1 chunks alternate pools A, B, A, ...
                if nbb == 2:
                    pools = [(psumA, "a"), (psumB, "b")]
                else:
                    pools = [[(psumA, "a"), (psumB, "b")][n512 % 2]]
                    n512 += 1
                psums = [
                    [pool.tile([P, BBLK], f32, tag=f"p{pc}{us}",
                               name=f"ps{ch}_{us}_{s}")
                     for s, (pool, pc) in enumerate(pools)]
                    for us in range(NUS)
                ]

                for kt in range(KT):
                    if ch == 0:
                        if kt >= 16:
                            # W tail free-runs on the HWDGE rings (proven
                            # ~115 GB/s combined; fully resident by ~180us)
                            weng = nc.sync if kt % 2 == 0 else nc.scalar
                            weng.dma_start(wres[kt - 16][:], w_d[kt - 16])
                        seg, col0 = kt, 0
                    else:
                        seg, col0 = kt // 2, (kt % 2) * BBLK
                    if (ch, seg) not in prefetched:
                        x_issue(ch, seg)
                    xs = prefetched[(ch, seg)]
                    if ch == 0 or kt % 2 == 1 or kt == KT - 1:
                        del prefetched[(ch, seg)]  # last use of this tile
                    # first k-tile after a boundary: touch the banks in
                    # the order the previous chunk's casts free them
                    us_order = [0, 2, 1, 3] if (kt == 0 and ch > 0) \
                        else range(NUS)
                    if kt < 4:
                        wsl = whp[kt // 2][:, (kt % 2) * U_SH:
                                           (kt % 2 + 1) * U_SH]
                    elif kt < 16:
                        g, jj = (kt - 4) // 4, (kt - 4) % 4
                        wsl = whq[g][:, jj * U_SH:(jj + 1) * U_SH]
                    else:
                        wsl = wres[kt - 16][:]
                    for us in us_order:
                        for s in range(nbb):
                            nc.tensor.matmul(
                                psums[us][s][:],
                                wsl[:, us * P:(us + 1) * P],
                                xs[:, col0 + s * BBLK:
                                       col0 + (s + 1) * BBLK],
                                start=(kt == 0),
                                stop=(kt == KT - 1),
                            )

                # Prefetch next chunk's first x pairs on gpsimd BEFORE the
                # drain -- the casts block sync/scalar on matmul sems, and
                # the gpsimd stream has no such waits.
                if ch + 1 < len(CHUNKS):
                    for pp in range(3):
                        x_issue(ch + 1, pp)

                # Drain: cast PSUM -> one stage tile; vector does us 0-1,
                # scalar does us 2-3 in parallel; pool-A banks (which the
                # next chunk needs first) are cast before pool-B banks.
                st = spool.tile([P, NUS * nbb * BBLK], bf16,
                                tag="st0" if ch == 0 else "st_s",
                                name=f"st{ch}")
                for s in range(nbb):          # s=0 (pool A) first
                    for us in range(NUS):
                        dst = st[:, (us * nbb + s) * BBLK:
                                 (us * nbb + s + 1) * BBLK]
                        if us < 2:
                            nc.vector.tensor_copy(dst, psums[us][s][:])
                        else:
                            nc.scalar.copy(dst, psums[us][s][:])
                if ch == len(CHUNKS) - 1:
                    # final chunk: quarter DMAs, each right after its cast,
                    # so the last (smallest) transfer finishes sooner
                    qw = nbb * BBLK
                    for us in range(NUS):
                        eng = nc.sync if us < 2 else nc.scalar
                        eng.dma_start(o_d[ch][:, us:us + 1],
                                      st[:, us * qw:(us + 1) * qw])
                else:
                    half = 2 * nbb * BBLK
                    nc.sync.dma_start(o_d[ch][:, 0:2], st[:, :half])
                    nc.scalar.dma_start(o_d[ch][:, 2:4], st[:, half:])

    nc.compile()
    return nc


def _get_nc():
    if "nc" not in _NC_CACHE:
        _NC_CACHE["nc"] = _build_nc()
    return _NC_CACHE["nc"]


def kernel(x, kernel_vector, bias, nonzero_ind):
    global LAST_RESULT
    from concourse.bass_utils import run_bass_kernel_spmd

    x = np.asarray(x, dtype=np.float32)
    kernel_vector = np.asarray(kernel_vector, dtype=np.float32)
    bias = np.asarray(bias, dtype=np.float32)
    nonzero_ind = np.asarray(nonzero_ind)

    nc = _get_nc()

    # Host scatter: dense weights [KT*P, U] fp16 (rows >= 20000 stay zero).
    rows = nonzero_ind[:, 0].astype(np.int64)
    cols = nonzero_ind[:, 1].astype(np.int64)
    w_full = np.zeros(KT * P * U, np.float32)
    np.add.at(w_full, rows * U + cols, kernel_vector)
    w_full = w_full.reshape(KT * P, U).astype(np.float16)

    # x^T padded to KPAD rows, fp16.
    x16 = x.astype(np.float16)
    xt = np.zeros((KPAD, B), np.float16)
    xt[:K] = x16.T

    xt_chunks = []
    b0 = 0
    for ch, (bch, nbb) in enumerate(CHUNKS):
        if ch == 0:
            xc = xt[:KT * P, b0:b0 + bch].reshape(KT, P, bch)
            xt_chunks.append(np.ascontiguousarray(xc))
        else:
            xc = (xt[:, b0:b0 + bch].reshape(NXP, 2, P, bch)
                  .transpose(0, 2, 1, 3))
            xt_chunks.append(np.ascontiguousarray(xc.reshape(NXP, P, 2 * bch)))
        b0 += bch

    in_maps = []
    for c in range(USPLIT):
        wc = w_full[:, c * U_SH:(c + 1) * U_SH]
        w_hp = np.ascontiguousarray(
            wc[:4 * P].reshape(2, 2, P, U_SH).transpose(0, 2, 1, 3)
            .reshape(2, P, 2 * U_SH))
        w_hq = np.ascontiguousarray(
            wc[4 * P:16 * P].reshape(3, 4, P, U_SH).transpose(0, 2, 1, 3)
            .reshape(3, P, 4 * U_SH))
        w_sh = np.ascontiguousarray(
            wc[16 * P:].reshape(KT - 16, P, U_SH))
        m = {"w_sh": w_sh, "w_hp": w_hp, "w_hq": w_hq}
        for ch in range(len(CHUNKS)):
            m[f"xt{ch}"] = xt_chunks[ch]
        in_maps.append(m)

    kwargs = {}
    if TRACE:
        kwargs = dict(trace=True, trace_cores=list(range(8)))
    res = run_bass_kernel_spmd(nc, in_maps, core_ids=list(range(8)), **kwargs)
    LAST_RESULT = res

    out = np.empty((B, U), np.float32)
    for c in range(USPLIT):
        b0 = 0
        for ch, (bch, nbb) in enumerate(CHUNKS):
            # [P, NUS, nbb*BBLK] -> [nbb, BBLK, NUS, P] -> [bch, U_SH]
            blk = (
                res.results[c][f"o{ch}"]
                .astype(np.float32)
                .reshape(P, NUS, nbb, BBLK)
                .transpose(2, 3, 1, 0)
                .reshape(bch, U_SH)
            )
            out[b0:b0 + bch, c * U_SH:(c + 1) * U_SH] = blk
            b0 += bch
    out += bias[None, :]
    np.tanh(out, out=out)
    return out
